# revision 3
# baseline (speedup 1.0000x reference)
"""Chamfer distance loss kernel for Trainium2 (8 NeuronCores).

Problem: B=4, N=8192, C=3. loss = mean_i min_j d[i,j] + mean_j min_i d[i,j]
over per-batch 8192x8192 squared-distance matrices.

v2 strategy — certified banded KNN + wide-outlier sweep (exact):
  - Host sorts both point sets by z. For each point, an upper bound on its
    NN distance (min over a 512-point subsample) converts to a rank
    interval via searchsorted; if the interval fits inside the fixed band
    it is CERTIFIED that the banded sweep finds its exact NN. Points that
    fail the cert go to full-sweep "wide" passes (capacity-padded).
  - Per core (2 cores per batch, rows split in half): 32 banded row-blocks
    [128 x (1536 band + 256 wide-cols)] + 1 wide-row block [128 x 8192].
    ~8.4M distance evals/core vs 33.5M for the dense matrix.
  - All 8 cores share one NEFF; per-core band offsets are made static by
    shipping a pre-shifted padded x-window (xwin) per core, with dummy
    columns (norm=1e30) outside the valid range.
  - PE computes d = lhsT.T @ rhs with hi/lo bf16-split augmentation
    (K=13) for near-fp32 distances; ScalarE drains PSUM f32 -> SBUF bf16;
    VectorE does row-min (tensor_scalar accum, 4x) and col-min
    (tensor_tensor min, 2x).
"""

import numpy as np

N_CORES = 8
P = 128
W = 704          # band half-width in sorted-rank space
BAND = 1536      # banded columns per row-block (3 x 512)
CAPX = 256       # wide x-hat columns per batch (padded capacity)
CAPT = 128       # wide target rows per core (padded capacity)
NB = 32          # banded row-blocks per core
NFULL = 8192
HALF = 4096
XWIN = (NB - 1) * P + BAND  # 5504
SUB = 512        # NN-bound subsample size
FLT_BIG = 3.0e38
PAD_NORM = 1.0e30
K_AUG = 13
TILE_FD = BAND + CAPX  # 1792
WR_CHUNKS = [1792, 1792, 1792, 1792, 1024]  # wide-row column chunks
TIMING_REPS = 5

_NC_CACHE = {}


def _build(reps):
    import concourse.bacc as bacc
    import concourse.mybir as mybir
    from concourse.tile import TileContext
    from contextlib import ExitStack

    f32 = mybir.dt.float32
    bf16 = mybir.dt.bfloat16
    MIN = mybir.AluOpType.min
    COPY = mybir.ActivationFunctionType.Copy

    nc = bacc.Bacc(None, target_bir_lowering=False)

    aug_t_d = nc.dram_tensor("aug_t", [K_AUG, HALF], bf16, kind="ExternalInput")
    xwin_d = nc.dram_tensor("xwin", [K_AUG, XWIN], bf16, kind="ExternalInput")
    augxf_d = nc.dram_tensor("aug_xf", [K_AUG, NFULL], bf16, kind="ExternalInput")
    aug_wt_d = nc.dram_tensor("aug_wt", [K_AUG, CAPT], bf16, kind="ExternalInput")
    aug_wx_d = nc.dram_tensor("aug_wx", [K_AUG, CAPX], bf16, kind="ExternalInput")

    rowmin_d = nc.dram_tensor("rowmin", [P, NB + len(WR_CHUNKS)], f32,
                              kind="ExternalOutput")
    colacc_d = nc.dram_tensor("colacc", [P, XWIN], bf16, kind="ExternalOutput")
    colaccw_d = nc.dram_tensor("colaccw", [P, CAPX], bf16, kind="ExternalOutput")
    widerow_d = nc.dram_tensor("widerow", [P, NFULL], bf16, kind="ExternalOutput")

    with TileContext(nc) as tc, ExitStack() as ctx:
        singles = ctx.enter_context(tc.tile_pool(name="singles", bufs=1))
        psum_pool = ctx.enter_context(
            tc.tile_pool(name="psum_pool", bufs=2, space="PSUM")
        )
        dpool = ctx.enter_context(tc.tile_pool(name="dpool", bufs=3))
        spool = ctx.enter_context(tc.tile_pool(name="spool", bufs=2))

        aug_t_sb = singles.tile([K_AUG, HALF], bf16)
        xwin_sb = singles.tile([K_AUG, XWIN], bf16)
        augxf_sb = singles.tile([K_AUG, NFULL], bf16)
        aug_wt_sb = singles.tile([K_AUG, CAPT], bf16)
        aug_wx_sb = singles.tile([K_AUG, CAPX], bf16)
        nc.sync.dma_start(out=aug_t_sb, in_=aug_t_d[:, :])
        nc.sync.dma_start(out=xwin_sb, in_=xwin_d[:, :])
        nc.sync.dma_start(out=augxf_sb, in_=augxf_d[:, :])
        nc.sync.dma_start(out=aug_wt_sb, in_=aug_wt_d[:, :])
        nc.sync.dma_start(out=aug_wx_sb, in_=aug_wx_d[:, :])

        colacc = singles.tile([P, XWIN], bf16)
        colaccw = singles.tile([P, CAPX], bf16)
        widerow = singles.tile([P, NFULL], bf16)
        rowmin = singles.tile([P, NB + len(WR_CHUNKS)], f32)

        for rep in range(reps):
            nc.vector.memset(colacc, FLT_BIG)
            nc.vector.memset(colaccw, FLT_BIG)
            for b in range(NB):
                psum = psum_pool.tile([P, TILE_FD], f32, tag="ps",
                                      name=f"ps_{rep}_{b}")
                lhsT = aug_t_sb[:, b * P : (b + 1) * P]
                for k in range(BAND // 512):
                    nc.tensor.matmul(
                        psum[:, k * 512 : (k + 1) * 512],
                        lhsT=lhsT,
                        rhs=xwin_sb[:, b * P + k * 512 : b * P + (k + 1) * 512],
                        start=True,
                        stop=True,
                    )
                nc.tensor.matmul(
                    psum[:, BAND:TILE_FD],
                    lhsT=lhsT,
                    rhs=aug_wx_sb[:, :],
                    start=True,
                    stop=True,
                )
                dtile = dpool.tile([P, TILE_FD], bf16, tag="dt",
                                   name=f"dt_{rep}_{b}")
                nc.scalar.activation(dtile, psum, COPY)
                scr = spool.tile([P, TILE_FD], bf16, tag="scr",
                                 name=f"scr_{rep}_{b}")
                nc.vector.tensor_scalar(
                    scr, dtile, FLT_BIG, None, op0=MIN, op1=MIN,
                    accum_out=rowmin[:, b : b + 1],
                )
                nc.vector.tensor_tensor(
                    colacc[:, b * P : b * P + BAND],
                    colacc[:, b * P : b * P + BAND],
                    dtile[:, 0:BAND],
                    MIN,
                )
                nc.vector.tensor_tensor(
                    colaccw, colaccw, dtile[:, BAND:TILE_FD], MIN
                )
            # wide-row pass: CAPT full rows x 8192 cols
            col = 0
            for c, width in enumerate(WR_CHUNKS):
                psum = psum_pool.tile([P, TILE_FD], f32, tag="ps",
                                      name=f"psw_{rep}_{c}")
                off = 0
                while off < width:
                    wmm = min(512, width - off)
                    nc.tensor.matmul(
                        psum[:, off : off + wmm],
                        lhsT=aug_wt_sb[:, :],
                        rhs=augxf_sb[:, col + off : col + off + wmm],
                        start=True,
                        stop=True,
                    )
                    off += wmm
                nc.scalar.activation(
                    widerow[:, col : col + width], psum[:, 0:width], COPY
                )
                scr = spool.tile([P, TILE_FD], bf16, tag="scr",
                                 name=f"scrw_{rep}_{c}")
                nc.vector.tensor_scalar(
                    scr[:, 0:width], widerow[:, col : col + width],
                    FLT_BIG, None, op0=MIN, op1=MIN,
                    accum_out=rowmin[:, NB + c : NB + c + 1],
                )
                col += width
            if rep == reps - 1:
                nc.sync.dma_start(out=rowmin_d[:, :], in_=rowmin)
                nc.sync.dma_start(out=colacc_d[:, :], in_=colacc)
                nc.sync.dma_start(out=colaccw_d[:, :], in_=colaccw)
                nc.sync.dma_start(out=widerow_d[:, :], in_=widerow)

    return nc


def _get_nc(reps):
    if reps not in _NC_CACHE:
        nc = _build(reps)
        nc.compile()
        _NC_CACHE[reps] = nc
    return _NC_CACHE[reps]


def _split_hi_lo(v):
    import ml_dtypes

    hi = v.astype(ml_dtypes.bfloat16)
    lo = (v - hi.astype(np.float32)).astype(ml_dtypes.bfloat16)
    return hi, lo


def _make_aug(t, x):
    """t: [R,3] f32, x: [N,3] f32 -> (aug_t [13,R] bf16, aug_x [13,N] bf16).

    d = sum_k aug_t[k].T * aug_x[k]:
      k0-2 : hi_t  *  hi_w      (w = -2x)
      k3-5 : lo_t  *  hi_w
      k6-8 : hi_t  *  lo_w
      k9   : nth   *  1         (nt = |t|^2 = nth + ntl)
      k10  : ntl   *  1
      k11  : 1     *  nxh       (nx = |x|^2 = nxh + nxl)
      k12  : 1     *  nxl
    """
    import ml_dtypes

    bf = ml_dtypes.bfloat16
    R = t.shape[0]
    N = x.shape[0]
    w = -2.0 * x
    ht, lt = _split_hi_lo(t.T)  # [3, R]
    hw, lw = _split_hi_lo(w.T)  # [3, N]
    nt = (t.astype(np.float64) ** 2).sum(1).astype(np.float32)
    nx = (x.astype(np.float64) ** 2).sum(1).astype(np.float32)
    nth, ntl = _split_hi_lo(nt)
    nxh, nxl = _split_hi_lo(nx)

    aug_t = np.empty((K_AUG, R), bf)
    aug_t[0:3] = ht
    aug_t[3:6] = lt
    aug_t[6:9] = ht
    aug_t[9] = nth
    aug_t[10] = ntl
    aug_t[11] = bf(1.0)
    aug_t[12] = bf(1.0)

    aug_x = np.empty((K_AUG, N), bf)
    aug_x[0:3] = hw
    aug_x[3:6] = hw
    aug_x[6:9] = lw
    aug_x[9] = bf(1.0)
    aug_x[10] = bf(1.0)
    aug_x[11] = nxh
    aug_x[12] = nxl
    return aug_t, aug_x


def _pad_col():
    """Augmented x-column whose distance to any target is ~1e30."""
    import ml_dtypes

    bf = ml_dtypes.bfloat16
    col = np.zeros((K_AUG, 1), bf)
    col[9] = bf(1.0)
    col[10] = bf(1.0)
    col[11] = bf(PAD_NORM)
    return col


def _nn_bound(a, bsub):
    """Upper bound (f32-safe) on distance from each a[i] to its NN in the
    full set bsub was sampled from: exact min distance to bsub + slack."""
    a2 = (a.astype(np.float32) ** 2).sum(1)
    b2 = (bsub.astype(np.float32) ** 2).sum(1)
    d = a2[:, None] + b2[None, :] - 2.0 * (a @ bsub.T)
    d = np.maximum(d.min(1), 0.0)
    return np.sqrt(d + 1e-4) * 1.0001


def _prep_batch(t, x):
    """Sort, certify, and build kernel inputs for one batch."""
    ot = np.argsort(t[:, 2], kind="stable")
    ox = np.argsort(x[:, 2], kind="stable")
    ts, xs = t[ot], x[ox]
    zt = ts[:, 2].astype(np.float64)
    zx = xs[:, 2].astype(np.float64)
    N = NFULL
    stride = N // SUB
    sub_idx = np.arange(SUB) * stride + stride // 2
    rt = _nn_bound(ts, xs[sub_idx])
    rx = _nn_bound(xs, ts[sub_idx])

    blk = np.arange(N) // P
    lo_x = np.searchsorted(zx, zt - rt)
    hi_x = np.searchsorted(zx, zt + rt, side="right")
    viol_t = np.maximum(blk * P - W - lo_x, hi_x - (blk * P + P + W))
    flag_t = viol_t > 0

    j = np.arange(N)
    lo_t = np.searchsorted(zt, zx - rx)
    hi_t = np.searchsorted(zt, zx + rx, side="right")
    blo = lo_t // P
    bhi = np.maximum(hi_t - 1, lo_t) // P
    viol_x = np.maximum(P * bhi - W - j, j - (P * blo + P - 1 + W))
    flag_x = viol_x > 0

    wt_all = np.where(flag_t)[0]
    if len(wt_all) > 2 * CAPT:
        wt_all = wt_all[np.argsort(-viol_t[wt_all])][: 2 * CAPT]
        wt_all.sort()
    wx_idx = np.where(flag_x)[0]
    if len(wx_idx) > CAPX:
        wx_idx = wx_idx[np.argsort(-viol_x[wx_idx])][:CAPX]
        wx_idx.sort()

    aug_tf, aug_xf = _make_aug(ts, xs)
    pad = _pad_col()

    # xwin per half: batch x-rank cols [HALF*h - W, HALF*h - W + XWIN)
    xwins = []
    for h in range(2):
        xwin = np.repeat(pad, XWIN, axis=1)
        lo = HALF * h - W
        s0 = max(0, -lo)
        s1 = min(XWIN, N - lo)
        xwin[:, s0:s1] = aug_xf[:, lo + s0 : lo + s1]
        xwins.append(xwin)

    aug_wx = np.repeat(pad, CAPX, axis=1)
    aug_wx[:, : len(wx_idx)] = aug_xf[:, wx_idx]

    # assign wide t rows to the 2 cores (cap CAPT each, cross-spill ok)
    half_of = (wt_all // HALF).astype(int)
    order = np.argsort(half_of, kind="stable")  # h=0 rows first
    wt_sorted = wt_all[order]
    n0 = int((half_of == 0).sum())
    core_rows = [[], []]
    for i, r in enumerate(wt_sorted):
        pref = 0 if i < n0 else 1
        if len(core_rows[pref]) < CAPT:
            core_rows[pref].append(int(r))
        else:
            core_rows[1 - pref].append(int(r))
    aug_wts, wt_maps = [], []
    for h in range(2):
        rows = core_rows[h]
        pad_rank = HALF * h  # any valid row; dup is harmless
        full = rows + [pad_rank] * (CAPT - len(rows))
        aug_wts.append(np.ascontiguousarray(aug_tf[:, full]))
        wt_maps.append(np.asarray(full))

    in_maps = []
    for h in range(2):
        in_maps.append({
            "aug_t": np.ascontiguousarray(aug_tf[:, HALF * h : HALF * (h + 1)]),
            "xwin": np.ascontiguousarray(xwins[h]),
            "aug_xf": aug_xf,
            "aug_wt": aug_wts[h],
            "aug_wx": aug_wx,
        })
    meta = {"wx_idx": wx_idx, "wt_maps": wt_maps}
    return in_maps, meta


def _combine_batch(res0, res1, meta):
    """res*: dicts of outputs for the 2 cores of one batch -> (sum_d1, sum_d2)."""
    N = NFULL
    d1 = np.full(N, np.inf)
    d2 = np.full(N, np.inf)
    for h, r in enumerate((res0, res1)):
        rowmin = np.asarray(r["rowmin"]).astype(np.float64)  # [P, NB+5]
        # banded row-mins: block bl, partition p -> rank HALF*h + 128*bl + p
        banded = rowmin[:, :NB].T.reshape(-1)  # [NB*P] rank-ordered
        lo = HALF * h
        d1[lo : lo + HALF] = np.minimum(d1[lo : lo + HALF], banded)
        # wide rows
        wr_min = rowmin[:, NB:].min(axis=1)  # [P]
        wt_map = meta["wt_maps"][h]
        np.minimum.at(d1, wt_map, wr_min)
        # col mins
        colacc = np.asarray(r["colacc"]).astype(np.float64).min(axis=0)  # [XWIN]
        base = HALF * h - W
        v0 = max(0, -base)
        v1 = min(XWIN, N - base)
        seg = slice(base + v0, base + v1)
        d2[seg] = np.minimum(d2[seg], colacc[v0:v1])
        widerow = np.asarray(r["widerow"]).astype(np.float64).min(axis=0)  # [N]
        d2 = np.minimum(d2, widerow)
        wx_idx = meta["wx_idx"]
        if len(wx_idx):
            caw = np.asarray(r["colaccw"]).astype(np.float64).min(axis=0)
            np.minimum.at(d2, wx_idx, caw[: len(wx_idx)])
    return float(d1.sum()), float(d2.sum())


def _prep(inputs):
    tp = np.ascontiguousarray(np.asarray(inputs["target_pos"], np.float32))
    xh = np.ascontiguousarray(np.asarray(inputs["x_hat"], np.float32))
    B = tp.shape[0]
    in_maps = []
    metas = []
    for b in range(B):
        ims, meta = _prep_batch(tp[b], xh[b])
        in_maps.extend(ims)
        metas.append(meta)
    return in_maps, metas


def _exec(in_maps, reps=1):
    nc = _get_nc(reps)
    from concourse.bass_utils import run_bass_kernel_spmd

    return run_bass_kernel_spmd(nc, in_maps, list(range(N_CORES)))


def _run(inputs, trace=False, reps=1, prep=None):
    in_maps, metas = prep if prep is not None else _prep(inputs)
    res = _exec(in_maps, reps=reps)
    s1 = 0.0
    s2 = 0.0
    for b in range(len(metas)):
        d1, d2 = _combine_batch(res.results[2 * b], res.results[2 * b + 1],
                                metas[b])
        s1 += d1
        s2 += d2
    loss = np.float32((s1 + s2) / (len(metas) * NFULL))
    return loss, res


def kernel(**inputs) -> np.ndarray:
    loss, _ = _run(inputs)
    return loss
